# revision 66
# baseline (speedup 1.0000x reference)
"""GroupedQueryAttention Trainium2 kernel (bf16, flipped PV, phase pipeline).

Sharding: 8 cores = 2 (batch) x 4 (KV-head groups). Each core computes, for
its batch b and its 2 KV heads (8 query heads = 512 q dims):
  qT = Wq_slice @ hidden[b].T             [512, S]   (dq on partitions)
  kT = Wk_slice @ hidden[b].T             [128, S]   + half-swapped copy kT_sw
  vT = Wv_slice @ hidden[b].T             [128, S] -> DMA-transposed v_tiles
  per head pair: scores sc[t,s] = k.q (psum f32); exp on Act -> bf16
  PV flipped: pv[s, d|Z] accumulated with rhs [v|1]: 65 streamed columns per
    key tile instead of 512 (matmul cost is output free-size only), with the
    8 accumulation chains run sequentially (one psum bank group at a time)
    over retained exp halves; halves combined on DVE in f32
  normalize on DVE with per-partition 1/Z; DMA-transpose to attn_T [dq, s]
  o_partial[s, :] = attn_T.T @ Wo_slice  (row-parallel)
Host sums the 4 partials per batch and adds bo.

Scheduling: a software-pipelined stream of 8-slot phases (QK+exp per slot,
previous phase's PV chains drained alongside, q/k/v/o-projection generators
interleaved as fillers) keeps the Act engine (~267us of exp, the secondary
bottleneck behind ~305us of PE) fed from ~15us onward; the final chunk is
split into two 256-query half-chunk phases (two key tiles packed per sc psum
tile to keep exp instructions at 1024 elements) so the closing o-projection
tail is halved.
"""

import numpy as np
import ml_dtypes

import concourse.mybir as mybir
import concourse.tile as tile
from concourse import bacc
from concourse.bass_utils import run_bass_kernel_spmd

P = 128
B, S, HID = 2, 2048, 2048
NH, G = 32, 8
HG = NH // G            # 4 query heads per KV head
D = HID // NH           # 64
NCORES = 8
GS = NCORES // B        # 4 head-group shards
DQ = HID // GS          # 512 q dims per core
DKV = G * D // GS       # 128 kv dims per core
CH = 512                # s-chunk width
NCH = S // CH           # 4
KT = HID // P           # 16 contraction tiles for projections
TT = S // P             # 16 key tiles
NPAIR = DQ // P         # 4 head pairs per core

f32 = mybir.dt.float32
bf16 = mybir.dt.bfloat16
EXPF = mybir.ActivationFunctionType.Exp
SCALE = 1.0 / float(np.sqrt(D))
DEBUG = False


def _emit(tc):
    nc = tc.nc
    ht = nc.dram_tensor("ht", [HID, S], bf16, kind="ExternalInput")
    # host pre-arranged for contiguous DMA rows (>=512B descriptors)
    wq = nc.dram_tensor("wq", [NPAIR, P, KT, P], bf16, kind="ExternalInput")
    wk = nc.dram_tensor("wk", [P, KT, DKV], bf16, kind="ExternalInput")
    wv = nc.dram_tensor("wv", [P, KT, DKV], bf16, kind="ExternalInput")
    wo = nc.dram_tensor("wo", [DQ, HID], bf16, kind="ExternalInput")
    bqd = nc.dram_tensor("bq", [DQ], f32, kind="ExternalInput")
    bkd = nc.dram_tensor("bk", [DKV], f32, kind="ExternalInput")
    bvd = nc.dram_tensor("bv", [DKV], f32, kind="ExternalInput")
    opart = nc.dram_tensor("opart", [S, HID], bf16, kind="ExternalOutput")

    consts = tc.alloc_tile_pool(name="consts", bufs=1)
    wpool = tc.alloc_tile_pool(name="wpool", bufs=1)
    persist = tc.alloc_tile_pool(name="persist", bufs=1)
    work = tc.alloc_tile_pool(name="work", bufs=2)
    expp = tc.alloc_tile_pool(name="expp", bufs=3)

    # DMAs in need-order: k path first, then first ht chunk, q pair 0, v.
    # Later ht chunks / wq pairs / wo are emitted inside the preamble below so
    # the greedy scheduler doesn't queue them ahead of critical small DMAs.
    bk_t = consts.tile([P, 1], f32)
    nc.sync.dma_start(out=bk_t[:], in_=bkd.rearrange("(p one) -> p one", p=P))
    bv_t = consts.tile([P, 1], f32)
    nc.sync.dma_start(out=bv_t[:], in_=bvd.rearrange("(p one) -> p one", p=P))
    bq_t = consts.tile([P, NPAIR], f32)
    nc.sync.dma_start(out=bq_t[:], in_=bqd.rearrange("(mt p) -> p mt", p=P))

    # dummy exp up-front: pulls the Exp bias const-AP DMA and the activation
    # table load ahead of the big weight DMAs in the queue
    warm = consts.tile([P, CH], bf16)
    nc.vector.memset(warm[:], 0.0)
    wexp = consts.tile([P, 1], bf16)
    nc.scalar.activation(out=wexp[:], in_=warm[:, 0:1], func=EXPF, scale=SCALE)

    wk_sb = wpool.tile([P, KT, DKV], bf16)
    nc.sync.dma_start(out=wk_sb[:], in_=wk[:])

    ht_sb = persist.tile([P, KT, S], bf16)
    ht_r = ht.rearrange("(kt p) s -> p kt s", p=P)
    wq_sb = wpool.tile([P, NPAIR, KT, P], bf16)
    nc.sync.dma_start(out=wq_sb[:, 0], in_=wq[0])
    for k4 in range(0, KT, 4):  # chunk 0 in kt-quarters: kproj starts early
        nc.sync.dma_start(out=ht_sb[:, k4:k4 + 4, 0:CH],
                          in_=ht_r[:, k4:k4 + 4, 0:CH])
    wv_sb = wpool.tile([P, KT, DKV], bf16)
    wo_sb = wpool.tile([P, NPAIR, HID], bf16)

    qT_sb = persist.tile([P, NPAIR, S], bf16)
    # kT_sb rows [0:64]=g0, [64:128]=g1; kT_sw has the halves swapped so
    # every (pair, head-parity) finds its kv head at the right partitions
    kT_sb = persist.tile([P, S], bf16)
    kT_sw = persist.tile([P, S], bf16)
    vT_sb = persist.tile([P, S], bf16)
    v_tiles = persist.tile([P, TT, 2, D + 1], bf16)
    attn_T = persist.tile([P, NPAIR, S], bf16)

    nc.vector.memset(v_tiles[:, :, :, D:D + 1], 1.0)

    with tc.tile_pool(name="ps", bufs=1, space="PSUM") as ps:
        # PE warm-up while DMAs stream in (ramps the p-state clock)
        wa = ps.tile([P, CH], f32, tag="aux", bufs=2, name="warm")
        for i in range(10):
            nc.tensor.matmul(wa[:], warm[:, 0:P], warm[:], start=True, stop=True)

        def kproj_gen(c):
            cs = slice(c * CH, (c + 1) * CH)
            ka = ps.tile([P, CH], f32, tag="aux", bufs=2, name=f"k{c}")
            for kt in range(KT):
                nc.tensor.matmul(ka[:], wk_sb[:, kt, :], ht_sb[:, kt, cs],
                                 start=(kt == 0), stop=(kt == KT - 1))
                if kt < KT - 1:
                    yield
            nc.vector.tensor_scalar_add(kT_sb[:, cs], ka[:], bk_t[:, 0:1])
            nc.sync.dma_start(out=kT_sw[D:P, cs], in_=kT_sb[0:D, cs])
            nc.sync.dma_start(out=kT_sw[0:D, cs], in_=kT_sb[D:P, cs])
            yield

        def vproj_gen(c):
            cs = slice(c * CH, (c + 1) * CH)
            va = ps.tile([P, CH], f32, tag="aux", bufs=2, name=f"v{c}")
            for kt in range(KT):
                nc.tensor.matmul(va[:], wv_sb[:, kt, :], ht_sb[:, kt, cs],
                                 start=(kt == 0), stop=(kt == KT - 1))
                if kt < KT - 1:
                    yield
            nc.vector.tensor_scalar_add(vT_sb[:, cs], va[:], bv_t[:, 0:1])
            yield
            for t in range(4 * c, 4 * (c + 1)):
                vtr = work.tile([P, P], bf16, tag="vtr", bufs=2)
                nc.sync.dma_start(out=vtr[:], in_=vT_sb[:, t * P:(t + 1) * P],
                                  transpose=True)
                for g in range(2):
                    nc.vector.tensor_copy(v_tiles[:, t, g, 0:D],
                                          vtr[:, g * D:(g + 1) * D])
            yield

        def qproj_gen(c, p):
            cs = slice(c * CH, (c + 1) * CH)
            qa = ps.tile([P, CH], f32, tag="aux", bufs=2, name=f"q{c}{p}")
            for kt in range(KT):
                nc.tensor.matmul(qa[:], wq_sb[:, p, kt, :],
                                 ht_sb[:, kt, cs], start=(kt == 0), stop=(kt == KT - 1))
                if kt < KT - 1:
                    yield
            nc.vector.tensor_scalar_add(qT_sb[:, p, cs], qa[:], bq_t[:, p:p + 1])
            yield

        def oproj_gen(c, stl):
            st = 4 * c + stl
            ss = slice(st * P, (st + 1) * P)
            for hc in range(HID // CH):
                hs = slice(hc * CH, (hc + 1) * CH)
                op = ps.tile([P, CH], f32, tag="aux", bufs=2, name=f"o{c}{stl}{hc}")
                for kt in range(NPAIR):
                    nc.tensor.matmul(op[:], attn_T[:, kt, ss], wo_sb[:, kt, hs],
                                     start=(kt == 0), stop=(kt == NPAIR - 1))
                    if kt < NPAIR - 1:
                        yield
                ostg = work.tile([P, CH], bf16, tag="ostg", bufs=4, name="ostg")
                nc.vector.tensor_copy(ostg[:], op[:])
                nc.sync.dma_start(out=opart[ss, hs], in_=ostg[:])
                yield

        fillers = []

        def drain(n):
            for _ in range(n):
                while fillers:
                    try:
                        next(fillers[0])
                        break
                    except StopIteration:
                        fillers.pop(0)
                else:
                    return

        HT = TT // 2                # 8 key tiles per half

        def k_lhs(p):
            # (even-head source rows 0:D, odd-head source rows D:P)
            if p < 2:           # kv head g0
                return kT_sb, kT_sw
            return kT_sw, kT_sb  # kv head g1

        def half_qk(c, p, half, exh):
            cs = slice(c * CH, (c + 1) * CH)
            ke, ko = k_lhs(p)
            for tl in range(HT):
                t = half * HT + tl
                ts_ = slice(t * P, (t + 1) * P)
                sc = ps.tile([P, 2, CH], f32, tag="sc", bufs=2)
                nc.tensor.matmul(sc[:, 0, :], ke[0:D, ts_],
                                 qT_sb[0:D, p, cs],
                                 tile_position=(0, 0), start=True, stop=True)
                nc.tensor.matmul(sc[:, 1, :], ko[D:P, ts_],
                                 qT_sb[D:P, p, cs],
                                 tile_position=(D, 0), start=True, stop=True)
                nc.scalar.activation(out=exh[:, tl, :, :], in_=sc[:],
                                     func=EXPF, scale=SCALE)
                yield

        def half_pv(c, p, half, exh, acc):
            # 8 sequential pv accumulation chains (one psum group at a time);
            # drained during the NEXT half's QK phase, when all exps are done.
            g = p // 2
            for h in range(2):
                for si in range(4):
                    pv = ps.tile([P, CH], f32, tag="pv", bufs=2)
                    for tl in range(HT):
                        t = half * HT + tl
                        nc.tensor.matmul(pv[:, 0:D + 1],
                                         exh[:, tl, h, si * P:(si + 1) * P],
                                         v_tiles[:, t, g, :],
                                         start=(tl == 0), stop=(tl == HT - 1))
                    if half == 0:
                        nc.vector.tensor_copy(acc[:, si, h, :], pv[:, 0:D + 1])
                    else:
                        nc.vector.tensor_add(acc[:, si, h, :],
                                             pv[:, 0:D + 1],
                                             acc[:, si, h, :])
                    yield

        def pair_finish(c, p, acc):
            # normalize by 1/Z (Z = column D of acc) on DVE, cast to bf16
            rz = work.tile([P, 4, 2, 1], f32, tag="rz", bufs=2)
            nc.vector.reciprocal(rz[:], acc[:, :, :, D:D + 1])
            an = work.tile([P, 4, P], bf16, tag="an", bufs=2)
            for si in range(4):
                for h in range(2):
                    nc.vector.tensor_scalar_mul(an[:, si, h * D:(h + 1) * D],
                                                acc[:, si, h, 0:D],
                                                rz[:, si, h, 0:1])
            for si in range(4):
                col = c * CH + si * P
                nc.sync.dma_start(out=attn_T[:, p, col:col + P],
                                  in_=an[:, si, :], transpose=True)

        # ---- 256-query half-chunk variants for the final chunk (smaller
        # o-proj tail). Two key-tiles are packed per sc psum tile so exp
        # instructions keep their 1024-element size. ----
        CH2 = CH // 2

        def qproj2_gen(hc, p):
            cs = slice(3 * CH + hc * CH2, 3 * CH + (hc + 1) * CH2)
            qa = ps.tile([P, CH], f32, tag="aux", bufs=2, name=f"q3{hc}{p}")
            for kt in range(KT):
                nc.tensor.matmul(qa[:, 0:CH2], wq_sb[:, p, kt, :],
                                 ht_sb[:, kt, cs], start=(kt == 0),
                                 stop=(kt == KT - 1))
                if kt < KT - 1:
                    yield
            nc.vector.tensor_scalar_add(qT_sb[:, p, cs], qa[:, 0:CH2],
                                        bq_t[:, p:p + 1])
            yield

        def hc_qk(hc, p, exh):
            cs = slice(3 * CH + hc * CH2, 3 * CH + (hc + 1) * CH2)
            ke, ko = k_lhs(p)
            for tl in range(HT):
                sc = ps.tile([P, 2, CH], f32, tag="sc", bufs=2)
                for j in range(2):
                    t = 2 * tl + j
                    ts_ = slice(t * P, (t + 1) * P)
                    js = slice(j * CH2, (j + 1) * CH2)
                    nc.tensor.matmul(sc[:, 0, js], ke[0:D, ts_],
                                     qT_sb[0:D, p, cs],
                                     tile_position=(0, 0), start=True, stop=True)
                    nc.tensor.matmul(sc[:, 1, js], ko[D:P, ts_],
                                     qT_sb[D:P, p, cs],
                                     tile_position=(D, 0), start=True, stop=True)
                nc.scalar.activation(out=exh[:, tl, :, :], in_=sc[:],
                                     func=EXPF, scale=SCALE)
                yield

        def hc_pv(hc, p, exh, acc):
            g = p // 2
            for h in range(2):
                for si in range(2):
                    pv = ps.tile([P, CH], f32, tag="pv", bufs=2)
                    for tl in range(HT):
                        for j in range(2):
                            t = 2 * tl + j
                            col = j * CH2 + si * P
                            nc.tensor.matmul(pv[:, 0:D + 1],
                                             exh[:, tl, h, col:col + P],
                                             v_tiles[:, t, g, :],
                                             start=(tl == 0 and j == 0),
                                             stop=(tl == HT - 1 and j == 1))
                    nc.vector.tensor_copy(acc[:, si, h, :], pv[:, 0:D + 1])
                    yield

        def hc_finish(hc, p, acc):
            rz = work.tile([P, 4, 2, 1], f32, tag="rz", bufs=2)
            nc.vector.reciprocal(rz[:, 0:2], acc[:, 0:2, :, D:D + 1])
            an = work.tile([P, 4, P], bf16, tag="an", bufs=2)
            for si in range(2):
                for h in range(2):
                    nc.vector.tensor_scalar_mul(an[:, si, h * D:(h + 1) * D],
                                                acc[:, si, h, 0:D],
                                                rz[:, si, h, 0:1])
            for si in range(2):
                col = 3 * CH + hc * CH2 + si * P
                nc.sync.dma_start(out=attn_T[:, p, col:col + P],
                                  in_=an[:, si, :], transpose=True)

        # ---- emission (DMAs interleaved in need-order so the greedy DMA
        # device doesn't starve the small ktrep/vtr copies) ----
        def run(g_):
            for _ in g_:
                pass

        def ht_chunk(c):
            # 4 kt-quarter pieces so later small DMAs aren't stuck behind 6us
            for k4 in range(0, KT, 4):
                nc.sync.dma_start(out=ht_sb[:, k4:k4 + 4, c * CH:(c + 1) * CH],
                                  in_=ht_r[:, k4:k4 + 4, c * CH:(c + 1) * CH])

        run(kproj_gen(0))
        run(qproj_gen(0, 0))
        nc.sync.dma_start(out=wv_sb[:], in_=wv[:])
        nc.sync.dma_start(out=wq_sb[:, 1], in_=wq[1])
        ht_chunk(1)
        wo_r = wo.rearrange("(kt p) m -> p kt m", p=P)

        # ---- pair (0,0): QK emission interleaved with the remaining
        # projections so the scheduler can start the Act engine early ----
        fillers.append(qproj_gen(0, 1))
        acc0 = work.tile([P, 4, 2, D + 1], f32, tag="acc", bufs=2)
        exh0 = expp.tile([P, HT, 2, CH], bf16, tag="exh", bufs=2)
        qk0 = half_qk(0, 0, 0, exh0)
        for _ in range(4):      # t0..3 need only k chunk 0
            next(qk0)
            drain(2)
        run(kproj_gen(1))
        for _ in range(4):      # t4..7 need k chunk 1
            next(qk0)
            drain(2)
        ht_chunk(2)
        run(vproj_gen(0))
        run(kproj_gen(2))
        exh1 = expp.tile([P, HT, 2, CH], bf16, tag="exh", bufs=2)
        qk1 = half_qk(0, 0, 1, exh1)
        for _ in range(4):      # t8..11 need k chunk 2
            next(qk1)
            drain(2)
        ht_chunk(3)
        run(kproj_gen(3))
        for _ in range(4):      # t12..15 need k chunk 3
            next(qk1)
            drain(2)
        run(vproj_gen(1))
        for _ in half_pv(0, 0, 0, exh0, acc0):   # needs v chunks 0,1
            drain(1)
        run(vproj_gen(2))
        run(vproj_gen(3))
        nc.sync.dma_start(out=wq_sb[:, 2], in_=wq[2])
        nc.sync.dma_start(out=wq_sb[:, 3], in_=wq[3])
        pend_pv = half_pv(0, 0, 1, exh1, acc0)   # drains during pair (0,1)
        pend_fin = (0, 0, acc0)

        # Remaining work as a uniform 8-slot phase pipeline. Each phase:
        # pre-hooks (filler appends / DMA emissions), 8 QK+exp slots draining
        # the previous phase's PV chains, then its own PV becomes pending.
        # Fillers are placed so no region is oversubscribed and the final
        # tail is just two 128-row o-proj tiles.
        phases = []

        # each phase = (hooks_fn, alloc_fn): hooks (filler appends / DMAs) run
        # at the phase's original position; alloc (tiles + QK generator) may
        # be called early so the next phase's first QK can be pre-emitted
        def reg_pair(c, p, hooks0, hooks1):
            acc = [None]

            def mk(half, hooks):
                def hooks_fn():
                    for h_ in hooks:
                        h_()

                def alloc():
                    if half == 0:
                        acc[0] = work.tile([P, 4, 2, D + 1], f32, tag="acc",
                                           bufs=2, name=f"acc{c}{p}")
                    exh = expp.tile([P, HT, 2, CH], bf16, tag="exh", bufs=2,
                                    name=f"exh{c}{p}{half}")
                    return (half_qk(c, p, half, exh),
                            lambda: half_pv(c, p, half, exh, acc[0]),
                            (c, p, acc[0]) if half == 1 else None)
                return (hooks_fn, alloc)
            phases.append(mk(0, hooks0))
            phases.append(mk(1, hooks1))

        def hc_pair(hc, p, hooks):
            def hooks_fn():
                for h_ in hooks:
                    h_()

            def alloc():
                acc = work.tile([P, 4, 2, D + 1], f32, tag="acc", bufs=2,
                                name=f"acch{hc}{p}")
                exh = expp.tile([P, HT, 2, CH], bf16, tag="exh", bufs=2,
                                name=f"exhh{hc}{p}")
                return (hc_qk(hc, p, exh),
                        lambda: hc_pv(hc, p, exh, acc),
                        ("hc", hc, p, acc))
            phases.append((hooks_fn, alloc))

        def addf(g_):
            return lambda: fillers.append(g_)

        def dma(out, in_):
            return lambda: nc.sync.dma_start(out=out, in_=in_)

        reg_pair(0, 1, [addf(qproj_gen(0, 2)), addf(qproj_gen(0, 3))], [])
        reg_pair(0, 2, [addf(qproj_gen(1, 0)), addf(qproj_gen(1, 1)),
                        dma(wo_sb[:, 0], wo_r[:, 0]),
                        dma(wo_sb[:, 1], wo_r[:, 1])], [])
        reg_pair(0, 3, [addf(qproj_gen(1, 2)), addf(qproj_gen(1, 3)),
                        dma(wo_sb[:, 2], wo_r[:, 2]),
                        dma(wo_sb[:, 3], wo_r[:, 3])], [])
        for p_ in range(NPAIR):
            reg_pair(1, p_, [addf(qproj_gen(2, p_))], [addf(oproj_gen(0, p_))])
        for p_ in range(NPAIR):
            q2 = [addf(qproj2_gen(p_ // 2, 2 * (p_ % 2))),
                  addf(qproj2_gen(p_ // 2, 2 * (p_ % 2) + 1))]
            reg_pair(2, p_, q2, [addf(oproj_gen(1, p_))])
        # fin(c,3) only fires after the NEXT phase's slot loop, so oproj
        # fillers for a chunk start two phases after its last pair
        for p_ in range(NPAIR):
            hc_pair(0, p_, [addf(oproj_gen(2, p_ - 1))] if p_ >= 1 else [])
        hc_hooks = [[addf(oproj_gen(2, 3))], [addf(oproj_gen(3, 0))],
                    [addf(oproj_gen(3, 1))], []]
        for p_ in range(NPAIR):
            hc_pair(1, p_, hc_hooks[p_])

        def fire_fin():
            if pend_fin[0] == "hc":
                hc_finish(*pend_fin[1:])
            else:
                pair_finish(*pend_fin)

        cur = None
        pre_emitted = 0
        for i, (hooks_fn, alloc_fn) in enumerate(phases):
            hooks_fn()
            if cur is None:
                cur = alloc_fn()
                pre_emitted = 0
            qk, pv_factory, fin = cur
            nxt = None
            for tl in range(pre_emitted, HT):
                next(qk)
                if pend_pv is not None:
                    if next(pend_pv, StopIteration) is StopIteration:
                        pend_pv = None
                        if pend_fin is not None:
                            fire_fin()
                            pend_fin = None
                    else:
                        drain(1)
                drain(2)
                if tl == HT - 2 and i + 1 < len(phases):
                    # pre-emit the next phase's first QKs so its exps can
                    # start the moment the Act engine finishes this phase
                    nxt = phases[i + 1][1]()
                    for _ in range(3):
                        next(nxt[0])
            if pend_pv is not None:
                for _ in pend_pv:
                    pass
                pend_pv = None
                if pend_fin is not None:
                    fire_fin()
                    pend_fin = None
            pend_pv = pv_factory()
            pend_fin = fin
            cur = nxt
            pre_emitted = 3 if nxt is not None else 0
        for _ in pend_pv:       # last phase's PV
            pass
        hc_finish(*pend_fin[1:])
        _left = 0
        while fillers:          # flush leftovers
            drain(1)
            _left += 1
        import os
        if os.getenv("KPRINT"):
            print(f"[emit] flushed {_left} leftover filler steps")
        for stl in (2, 3):
            for _ in oproj_gen(3, stl):
                pass

        if DEBUG:
            dbg = {
                "d_qT": qT_sb, "d_ktrepA": kT_sb, "d_ktrepB": kT_sw,
                "d_vT": vT_sb, "d_vtiles": v_tiles, "d_attnT": attn_T,
            }
            for name, t_ in dbg.items():
                dt_ = nc.dram_tensor(name, list(t_.shape), bf16,
                                     kind="ExternalOutput")
                nc.sync.dma_start(out=dt_[:], in_=t_[:])

    for pool in (expp, work, persist, wpool, consts):
        pool.release()


_NC_CACHE = None


def build_nc():
    global _NC_CACHE
    if _NC_CACHE is None:
        nc = bacc.Bacc("TRN2")
        with tile.TileContext(nc) as tc:
            _emit(tc)
        nc.compile()
        _NC_CACHE = nc
    return _NC_CACHE


def _bf16(a):
    return np.ascontiguousarray(np.asarray(a, dtype=np.float32)).astype(
        ml_dtypes.bfloat16)


def make_in_maps(hidden_state, Wq, bq, Wk, bk, Wv, bv, Wo):
    hidden_state = np.asarray(hidden_state, dtype=np.float32)
    Wq, Wk, Wv, Wo = (np.asarray(a, dtype=np.float32) for a in (Wq, Wk, Wv, Wo))
    bq, bk, bv = (np.asarray(a, dtype=np.float32) for a in (bq, bk, bv))
    htb = [_bf16(hidden_state[b].T) for b in range(B)]
    in_maps = []
    for core in range(NCORES):
        b, gs = divmod(core, GS)
        # wq: [HID, DQ] -> [NPAIR, P(part), KT, P(cols)]
        wqt = Wq[gs * DQ:(gs + 1) * DQ, :].T.reshape(KT, P, NPAIR, P)
        # wk/wv: [HID, DKV] -> [P(part), KT, DKV]
        wkt = Wk[gs * DKV:(gs + 1) * DKV, :].T.reshape(KT, P, DKV)
        wvt = Wv[gs * DKV:(gs + 1) * DKV, :].T.reshape(KT, P, DKV)
        in_maps.append({
            "ht": htb[b],
            "wq": _bf16(wqt.transpose(2, 1, 0, 3)),
            "wk": _bf16(wkt.transpose(1, 0, 2)),
            "wv": _bf16(wvt.transpose(1, 0, 2)),
            "wo": _bf16(Wo[:, gs * DQ:(gs + 1) * DQ].T),
            "bq": np.ascontiguousarray(bq[gs * DQ:(gs + 1) * DQ]),
            "bk": np.ascontiguousarray(bk[gs * DKV:(gs + 1) * DKV]),
            "bv": np.ascontiguousarray(bv[gs * DKV:(gs + 1) * DKV]),
        })
    return in_maps


def unshard(results, bo):
    bo = np.asarray(bo, dtype=np.float32)
    out = np.empty((B, S, HID), dtype=np.float32)
    for b in range(B):
        acc = np.zeros((S, HID), dtype=np.float64)
        for gs in range(GS):
            acc += np.asarray(results[b * GS + gs]["opart"], dtype=np.float32)
        out[b] = (acc + bo).astype(np.float32)
    return out


def kernel(hidden_state, attention_mask, Wq, bq, Wk, bk, Wv, bv, Wo, bo):
    # attention_mask is all-ones for this problem (fill: ones) -> identity.
    nc = build_nc()
    in_maps = make_in_maps(hidden_state, Wq, bq, Wk, bk, Wv, bv, Wo)
    res = run_bass_kernel_spmd(nc, in_maps, list(range(NCORES)))
    return unshard(res.results, bo)


# revision 67
# speedup vs baseline: 1.0032x; 1.0032x over previous
"""GroupedQueryAttention Trainium2 kernel (bf16, flipped PV, phase pipeline).

Sharding: 8 cores = 2 (batch) x 4 (KV-head groups). Each core computes, for
its batch b and its 2 KV heads (8 query heads = 512 q dims):
  qT = Wq_slice @ hidden[b].T             [512, S]   (dq on partitions)
  kT = Wk_slice @ hidden[b].T             [128, S]   + half-swapped copy kT_sw
  vT = Wv_slice @ hidden[b].T             [128, S] -> DMA-transposed v_tiles
  per head pair: scores sc[t,s] = k.q (psum f32); exp on Act -> bf16
  PV flipped: pv[s, d|Z] accumulated with rhs [v|1]: 65 streamed columns per
    key tile instead of 512 (matmul cost is output free-size only), with the
    8 accumulation chains run sequentially (one psum bank group at a time)
    over retained exp halves; halves combined on DVE in f32
  normalize on DVE with per-partition 1/Z; DMA-transpose to attn_T [dq, s]
  o_partial[s, :] = attn_T.T @ Wo_slice  (row-parallel)
Host sums the 4 partials per batch and adds bo.

Scheduling: a software-pipelined stream of 8-slot phases (QK+exp per slot,
previous phase's PV chains drained alongside, q/k/v/o-projection generators
interleaved as fillers) keeps the Act engine (~267us of exp, the secondary
bottleneck behind ~305us of PE) fed from ~15us onward; the final chunk is
split into two 256-query half-chunk phases (two key tiles packed per sc psum
tile to keep exp instructions at 1024 elements) so the closing o-projection
tail is halved.
"""

import numpy as np
import ml_dtypes

import concourse.mybir as mybir
import concourse.tile as tile
from concourse import bacc
from concourse.bass_utils import run_bass_kernel_spmd

P = 128
B, S, HID = 2, 2048, 2048
NH, G = 32, 8
HG = NH // G            # 4 query heads per KV head
D = HID // NH           # 64
NCORES = 8
GS = NCORES // B        # 4 head-group shards
DQ = HID // GS          # 512 q dims per core
DKV = G * D // GS       # 128 kv dims per core
CH = 512                # s-chunk width
NCH = S // CH           # 4
KT = HID // P           # 16 contraction tiles for projections
TT = S // P             # 16 key tiles
NPAIR = DQ // P         # 4 head pairs per core

f32 = mybir.dt.float32
bf16 = mybir.dt.bfloat16
EXPF = mybir.ActivationFunctionType.Exp
SCALE = 1.0 / float(np.sqrt(D))
DEBUG = False


def _emit(tc):
    nc = tc.nc
    ht = nc.dram_tensor("ht", [HID, S], bf16, kind="ExternalInput")
    # host pre-arranged for contiguous DMA rows (>=512B descriptors)
    wq = nc.dram_tensor("wq", [NPAIR, P, KT, P], bf16, kind="ExternalInput")
    wk = nc.dram_tensor("wk", [P, KT, DKV], bf16, kind="ExternalInput")
    wv = nc.dram_tensor("wv", [P, KT, DKV], bf16, kind="ExternalInput")
    wo = nc.dram_tensor("wo", [DQ, HID], bf16, kind="ExternalInput")
    bqd = nc.dram_tensor("bq", [DQ], f32, kind="ExternalInput")
    bkd = nc.dram_tensor("bk", [DKV], f32, kind="ExternalInput")
    bvd = nc.dram_tensor("bv", [DKV], f32, kind="ExternalInput")
    opart = nc.dram_tensor("opart", [S, HID], bf16, kind="ExternalOutput")

    consts = tc.alloc_tile_pool(name="consts", bufs=1)
    wpool = tc.alloc_tile_pool(name="wpool", bufs=1)
    persist = tc.alloc_tile_pool(name="persist", bufs=1)
    work = tc.alloc_tile_pool(name="work", bufs=2)
    expp = tc.alloc_tile_pool(name="expp", bufs=3)

    # DMAs in need-order: k path first, then first ht chunk, q pair 0, v.
    # Later ht chunks / wq pairs / wo are emitted inside the preamble below so
    # the greedy scheduler doesn't queue them ahead of critical small DMAs.
    bk_t = consts.tile([P, 1], f32)
    nc.sync.dma_start(out=bk_t[:], in_=bkd.rearrange("(p one) -> p one", p=P))
    bv_t = consts.tile([P, 1], f32)
    nc.sync.dma_start(out=bv_t[:], in_=bvd.rearrange("(p one) -> p one", p=P))
    bq_t = consts.tile([P, NPAIR], f32)
    nc.sync.dma_start(out=bq_t[:], in_=bqd.rearrange("(mt p) -> p mt", p=P))

    # dummy exp up-front: pulls the Exp bias const-AP DMA and the activation
    # table load ahead of the big weight DMAs in the queue
    warm = consts.tile([P, CH], bf16)
    nc.vector.memset(warm[:], 0.0)
    wexp = consts.tile([P, 1], bf16)
    nc.scalar.activation(out=wexp[:], in_=warm[:, 0:1], func=EXPF, scale=SCALE)

    wk_sb = wpool.tile([P, KT, DKV], bf16)
    nc.sync.dma_start(out=wk_sb[:], in_=wk[:])

    ht_sb = persist.tile([P, KT, S], bf16)
    ht_r = ht.rearrange("(kt p) s -> p kt s", p=P)
    wq_sb = wpool.tile([P, NPAIR, KT, P], bf16)
    nc.sync.dma_start(out=wq_sb[:, 0], in_=wq[0])
    for k4 in range(0, KT, 4):  # chunk 0 in kt-quarters: kproj starts early
        nc.sync.dma_start(out=ht_sb[:, k4:k4 + 4, 0:CH],
                          in_=ht_r[:, k4:k4 + 4, 0:CH])
    wv_sb = wpool.tile([P, KT, DKV], bf16)
    wo_sb = wpool.tile([P, NPAIR, HID], bf16)

    qT_sb = persist.tile([P, NPAIR, S], bf16)
    # kT_sb rows [0:64]=g0, [64:128]=g1; kT_sw has the halves swapped so
    # every (pair, head-parity) finds its kv head at the right partitions
    kT_sb = persist.tile([P, S], bf16)
    kT_sw = persist.tile([P, S], bf16)
    vT_sb = persist.tile([P, S], bf16)
    v_tiles = persist.tile([P, TT, 2, D + 1], bf16)
    attn_T = persist.tile([P, NPAIR, S], bf16)

    nc.vector.memset(v_tiles[:, :, :, D:D + 1], 1.0)

    with tc.tile_pool(name="ps", bufs=1, space="PSUM") as ps:
        # PE warm-up while DMAs stream in (ramps the p-state clock)
        wa = ps.tile([P, CH], f32, tag="aux", bufs=2, name="warm")
        for i in range(14):
            nc.tensor.matmul(wa[:], warm[:, 0:P], warm[:], start=True, stop=True)

        def kproj_gen(c):
            cs = slice(c * CH, (c + 1) * CH)
            ka = ps.tile([P, CH], f32, tag="aux", bufs=2, name=f"k{c}")
            for kt in range(KT):
                nc.tensor.matmul(ka[:], wk_sb[:, kt, :], ht_sb[:, kt, cs],
                                 start=(kt == 0), stop=(kt == KT - 1))
                if kt < KT - 1:
                    yield
            nc.vector.tensor_scalar_add(kT_sb[:, cs], ka[:], bk_t[:, 0:1])
            nc.sync.dma_start(out=kT_sw[D:P, cs], in_=kT_sb[0:D, cs])
            nc.sync.dma_start(out=kT_sw[0:D, cs], in_=kT_sb[D:P, cs])
            yield

        def vproj_gen(c):
            cs = slice(c * CH, (c + 1) * CH)
            va = ps.tile([P, CH], f32, tag="aux", bufs=2, name=f"v{c}")
            for kt in range(KT):
                nc.tensor.matmul(va[:], wv_sb[:, kt, :], ht_sb[:, kt, cs],
                                 start=(kt == 0), stop=(kt == KT - 1))
                if kt < KT - 1:
                    yield
            nc.vector.tensor_scalar_add(vT_sb[:, cs], va[:], bv_t[:, 0:1])
            yield
            for t in range(4 * c, 4 * (c + 1)):
                vtr = work.tile([P, P], bf16, tag="vtr", bufs=2)
                nc.sync.dma_start(out=vtr[:], in_=vT_sb[:, t * P:(t + 1) * P],
                                  transpose=True)
                for g in range(2):
                    nc.vector.tensor_copy(v_tiles[:, t, g, 0:D],
                                          vtr[:, g * D:(g + 1) * D])
            yield

        def qproj_gen(c, p):
            cs = slice(c * CH, (c + 1) * CH)
            qa = ps.tile([P, CH], f32, tag="aux", bufs=2, name=f"q{c}{p}")
            for kt in range(KT):
                nc.tensor.matmul(qa[:], wq_sb[:, p, kt, :],
                                 ht_sb[:, kt, cs], start=(kt == 0), stop=(kt == KT - 1))
                if kt < KT - 1:
                    yield
            nc.vector.tensor_scalar_add(qT_sb[:, p, cs], qa[:], bq_t[:, p:p + 1])
            yield

        def oproj_gen(c, stl):
            st = 4 * c + stl
            ss = slice(st * P, (st + 1) * P)
            for hc in range(HID // CH):
                hs = slice(hc * CH, (hc + 1) * CH)
                op = ps.tile([P, CH], f32, tag="aux", bufs=2, name=f"o{c}{stl}{hc}")
                for kt in range(NPAIR):
                    nc.tensor.matmul(op[:], attn_T[:, kt, ss], wo_sb[:, kt, hs],
                                     start=(kt == 0), stop=(kt == NPAIR - 1))
                    if kt < NPAIR - 1:
                        yield
                ostg = work.tile([P, CH], bf16, tag="ostg", bufs=4, name="ostg")
                nc.vector.tensor_copy(ostg[:], op[:])
                nc.sync.dma_start(out=opart[ss, hs], in_=ostg[:])
                yield

        fillers = []

        def drain(n):
            for _ in range(n):
                while fillers:
                    try:
                        next(fillers[0])
                        break
                    except StopIteration:
                        fillers.pop(0)
                else:
                    return

        HT = TT // 2                # 8 key tiles per half

        def k_lhs(p):
            # (even-head source rows 0:D, odd-head source rows D:P)
            if p < 2:           # kv head g0
                return kT_sb, kT_sw
            return kT_sw, kT_sb  # kv head g1

        def half_qk(c, p, half, exh):
            cs = slice(c * CH, (c + 1) * CH)
            ke, ko = k_lhs(p)
            for tl in range(HT):
                t = half * HT + tl
                ts_ = slice(t * P, (t + 1) * P)
                sc = ps.tile([P, 2, CH], f32, tag="sc", bufs=2)
                nc.tensor.matmul(sc[:, 0, :], ke[0:D, ts_],
                                 qT_sb[0:D, p, cs],
                                 tile_position=(0, 0), start=True, stop=True)
                nc.tensor.matmul(sc[:, 1, :], ko[D:P, ts_],
                                 qT_sb[D:P, p, cs],
                                 tile_position=(D, 0), start=True, stop=True)
                nc.scalar.activation(out=exh[:, tl, :, :], in_=sc[:],
                                     func=EXPF, scale=SCALE)
                yield

        def half_pv(c, p, half, exh, acc):
            # 8 sequential pv accumulation chains (one psum group at a time);
            # drained during the NEXT half's QK phase, when all exps are done.
            g = p // 2
            for h in range(2):
                for si in range(4):
                    pv = ps.tile([P, CH], f32, tag="pv", bufs=2)
                    for tl in range(HT):
                        t = half * HT + tl
                        nc.tensor.matmul(pv[:, 0:D + 1],
                                         exh[:, tl, h, si * P:(si + 1) * P],
                                         v_tiles[:, t, g, :],
                                         start=(tl == 0), stop=(tl == HT - 1))
                    if half == 0:
                        nc.vector.tensor_copy(acc[:, si, h, :], pv[:, 0:D + 1])
                    else:
                        nc.vector.tensor_add(acc[:, si, h, :],
                                             pv[:, 0:D + 1],
                                             acc[:, si, h, :])
                    yield

        def pair_finish(c, p, acc):
            # normalize by 1/Z (Z = column D of acc) on DVE, cast to bf16
            rz = work.tile([P, 4, 2, 1], f32, tag="rz", bufs=2)
            nc.vector.reciprocal(rz[:], acc[:, :, :, D:D + 1])
            an = work.tile([P, 4, P], bf16, tag="an", bufs=2)
            for si in range(4):
                for h in range(2):
                    nc.vector.tensor_scalar_mul(an[:, si, h * D:(h + 1) * D],
                                                acc[:, si, h, 0:D],
                                                rz[:, si, h, 0:1])
            for si in range(4):
                col = c * CH + si * P
                nc.sync.dma_start(out=attn_T[:, p, col:col + P],
                                  in_=an[:, si, :], transpose=True)

        # ---- 256-query half-chunk variants for the final chunk (smaller
        # o-proj tail). Two key-tiles are packed per sc psum tile so exp
        # instructions keep their 1024-element size. ----
        CH2 = CH // 2

        def qproj2_gen(hc, p):
            cs = slice(3 * CH + hc * CH2, 3 * CH + (hc + 1) * CH2)
            qa = ps.tile([P, CH], f32, tag="aux", bufs=2, name=f"q3{hc}{p}")
            for kt in range(KT):
                nc.tensor.matmul(qa[:, 0:CH2], wq_sb[:, p, kt, :],
                                 ht_sb[:, kt, cs], start=(kt == 0),
                                 stop=(kt == KT - 1))
                if kt < KT - 1:
                    yield
            nc.vector.tensor_scalar_add(qT_sb[:, p, cs], qa[:, 0:CH2],
                                        bq_t[:, p:p + 1])
            yield

        def hc_qk(hc, p, exh):
            cs = slice(3 * CH + hc * CH2, 3 * CH + (hc + 1) * CH2)
            ke, ko = k_lhs(p)
            for tl in range(HT):
                sc = ps.tile([P, 2, CH], f32, tag="sc", bufs=2)
                for j in range(2):
                    t = 2 * tl + j
                    ts_ = slice(t * P, (t + 1) * P)
                    js = slice(j * CH2, (j + 1) * CH2)
                    nc.tensor.matmul(sc[:, 0, js], ke[0:D, ts_],
                                     qT_sb[0:D, p, cs],
                                     tile_position=(0, 0), start=True, stop=True)
                    nc.tensor.matmul(sc[:, 1, js], ko[D:P, ts_],
                                     qT_sb[D:P, p, cs],
                                     tile_position=(D, 0), start=True, stop=True)
                nc.scalar.activation(out=exh[:, tl, :, :], in_=sc[:],
                                     func=EXPF, scale=SCALE)
                yield

        def hc_pv(hc, p, exh, acc):
            g = p // 2
            for h in range(2):
                for si in range(2):
                    pv = ps.tile([P, CH], f32, tag="pv", bufs=2)
                    for tl in range(HT):
                        for j in range(2):
                            t = 2 * tl + j
                            col = j * CH2 + si * P
                            nc.tensor.matmul(pv[:, 0:D + 1],
                                             exh[:, tl, h, col:col + P],
                                             v_tiles[:, t, g, :],
                                             start=(tl == 0 and j == 0),
                                             stop=(tl == HT - 1 and j == 1))
                    nc.vector.tensor_copy(acc[:, si, h, :], pv[:, 0:D + 1])
                    yield

        def hc_finish(hc, p, acc):
            rz = work.tile([P, 4, 2, 1], f32, tag="rz", bufs=2)
            nc.vector.reciprocal(rz[:, 0:2], acc[:, 0:2, :, D:D + 1])
            an = work.tile([P, 4, P], bf16, tag="an", bufs=2)
            for si in range(2):
                for h in range(2):
                    nc.vector.tensor_scalar_mul(an[:, si, h * D:(h + 1) * D],
                                                acc[:, si, h, 0:D],
                                                rz[:, si, h, 0:1])
            for si in range(2):
                col = 3 * CH + hc * CH2 + si * P
                nc.sync.dma_start(out=attn_T[:, p, col:col + P],
                                  in_=an[:, si, :], transpose=True)

        # ---- emission (DMAs interleaved in need-order so the greedy DMA
        # device doesn't starve the small ktrep/vtr copies) ----
        def run(g_):
            for _ in g_:
                pass

        def ht_chunk(c):
            # 4 kt-quarter pieces so later small DMAs aren't stuck behind 6us
            for k4 in range(0, KT, 4):
                nc.sync.dma_start(out=ht_sb[:, k4:k4 + 4, c * CH:(c + 1) * CH],
                                  in_=ht_r[:, k4:k4 + 4, c * CH:(c + 1) * CH])

        run(kproj_gen(0))
        run(qproj_gen(0, 0))
        nc.sync.dma_start(out=wv_sb[:], in_=wv[:])
        nc.sync.dma_start(out=wq_sb[:, 1], in_=wq[1])
        ht_chunk(1)
        wo_r = wo.rearrange("(kt p) m -> p kt m", p=P)

        # ---- pair (0,0): QK emission interleaved with the remaining
        # projections so the scheduler can start the Act engine early ----
        fillers.append(qproj_gen(0, 1))
        acc0 = work.tile([P, 4, 2, D + 1], f32, tag="acc", bufs=2)
        exh0 = expp.tile([P, HT, 2, CH], bf16, tag="exh", bufs=2)
        qk0 = half_qk(0, 0, 0, exh0)
        for _ in range(4):      # t0..3 need only k chunk 0
            next(qk0)
            drain(2)
        run(kproj_gen(1))
        for _ in range(4):      # t4..7 need k chunk 1
            next(qk0)
            drain(2)
        ht_chunk(2)
        run(vproj_gen(0))
        run(kproj_gen(2))
        exh1 = expp.tile([P, HT, 2, CH], bf16, tag="exh", bufs=2)
        qk1 = half_qk(0, 0, 1, exh1)
        for _ in range(4):      # t8..11 need k chunk 2
            next(qk1)
            drain(2)
        ht_chunk(3)
        run(kproj_gen(3))
        for _ in range(4):      # t12..15 need k chunk 3
            next(qk1)
            drain(2)
        run(vproj_gen(1))
        for _ in half_pv(0, 0, 0, exh0, acc0):   # needs v chunks 0,1
            drain(1)
        run(vproj_gen(2))
        run(vproj_gen(3))
        nc.sync.dma_start(out=wq_sb[:, 2], in_=wq[2])
        nc.sync.dma_start(out=wq_sb[:, 3], in_=wq[3])
        pend_pv = half_pv(0, 0, 1, exh1, acc0)   # drains during pair (0,1)
        pend_fin = (0, 0, acc0)

        # Remaining work as a uniform 8-slot phase pipeline. Each phase:
        # pre-hooks (filler appends / DMA emissions), 8 QK+exp slots draining
        # the previous phase's PV chains, then its own PV becomes pending.
        # Fillers are placed so no region is oversubscribed and the final
        # tail is just two 128-row o-proj tiles.
        phases = []

        # each phase = (hooks_fn, alloc_fn): hooks (filler appends / DMAs) run
        # at the phase's original position; alloc (tiles + QK generator) may
        # be called early so the next phase's first QK can be pre-emitted
        def reg_pair(c, p, hooks0, hooks1):
            acc = [None]

            def mk(half, hooks):
                def hooks_fn():
                    for h_ in hooks:
                        h_()

                def alloc():
                    if half == 0:
                        acc[0] = work.tile([P, 4, 2, D + 1], f32, tag="acc",
                                           bufs=2, name=f"acc{c}{p}")
                    exh = expp.tile([P, HT, 2, CH], bf16, tag="exh", bufs=2,
                                    name=f"exh{c}{p}{half}")
                    return (half_qk(c, p, half, exh),
                            lambda: half_pv(c, p, half, exh, acc[0]),
                            (c, p, acc[0]) if half == 1 else None)
                return (hooks_fn, alloc)
            phases.append(mk(0, hooks0))
            phases.append(mk(1, hooks1))

        def hc_pair(hc, p, hooks):
            def hooks_fn():
                for h_ in hooks:
                    h_()

            def alloc():
                acc = work.tile([P, 4, 2, D + 1], f32, tag="acc", bufs=2,
                                name=f"acch{hc}{p}")
                exh = expp.tile([P, HT, 2, CH], bf16, tag="exh", bufs=2,
                                name=f"exhh{hc}{p}")
                return (hc_qk(hc, p, exh),
                        lambda: hc_pv(hc, p, exh, acc),
                        ("hc", hc, p, acc))
            phases.append((hooks_fn, alloc))

        def addf(g_):
            return lambda: fillers.append(g_)

        def dma(out, in_):
            return lambda: nc.sync.dma_start(out=out, in_=in_)

        reg_pair(0, 1, [addf(qproj_gen(0, 2)), addf(qproj_gen(0, 3))], [])
        reg_pair(0, 2, [addf(qproj_gen(1, 0)), addf(qproj_gen(1, 1)),
                        dma(wo_sb[:, 0], wo_r[:, 0]),
                        dma(wo_sb[:, 1], wo_r[:, 1])], [])
        reg_pair(0, 3, [addf(qproj_gen(1, 2)), addf(qproj_gen(1, 3)),
                        dma(wo_sb[:, 2], wo_r[:, 2]),
                        dma(wo_sb[:, 3], wo_r[:, 3])], [])
        for p_ in range(NPAIR):
            reg_pair(1, p_, [addf(qproj_gen(2, p_))], [addf(oproj_gen(0, p_))])
        for p_ in range(NPAIR):
            q2 = [addf(qproj2_gen(p_ // 2, 2 * (p_ % 2))),
                  addf(qproj2_gen(p_ // 2, 2 * (p_ % 2) + 1))]
            reg_pair(2, p_, q2, [addf(oproj_gen(1, p_))])
        # fin(c,3) only fires after the NEXT phase's slot loop, so oproj
        # fillers for a chunk start two phases after its last pair
        for p_ in range(NPAIR):
            hc_pair(0, p_, [addf(oproj_gen(2, p_ - 1))] if p_ >= 1 else [])
        hc_hooks = [[addf(oproj_gen(2, 3))], [addf(oproj_gen(3, 0))],
                    [addf(oproj_gen(3, 1))], []]
        for p_ in range(NPAIR):
            hc_pair(1, p_, hc_hooks[p_])

        def fire_fin():
            if pend_fin[0] == "hc":
                hc_finish(*pend_fin[1:])
            else:
                pair_finish(*pend_fin)

        cur = None
        pre_emitted = 0
        for i, (hooks_fn, alloc_fn) in enumerate(phases):
            hooks_fn()
            if cur is None:
                cur = alloc_fn()
                pre_emitted = 0
            qk, pv_factory, fin = cur
            nxt = None
            for tl in range(pre_emitted, HT):
                next(qk)
                if pend_pv is not None:
                    if next(pend_pv, StopIteration) is StopIteration:
                        pend_pv = None
                        if pend_fin is not None:
                            fire_fin()
                            pend_fin = None
                    else:
                        drain(1)
                drain(2)
                if tl == HT - 2 and i + 1 < len(phases):
                    # pre-emit the next phase's first QKs so its exps can
                    # start the moment the Act engine finishes this phase
                    nxt = phases[i + 1][1]()
                    for _ in range(3):
                        next(nxt[0])
            if pend_pv is not None:
                for _ in pend_pv:
                    pass
                pend_pv = None
                if pend_fin is not None:
                    fire_fin()
                    pend_fin = None
            pend_pv = pv_factory()
            pend_fin = fin
            cur = nxt
            pre_emitted = 3 if nxt is not None else 0
        for _ in pend_pv:       # last phase's PV
            pass
        hc_finish(*pend_fin[1:])
        _left = 0
        while fillers:          # flush leftovers
            drain(1)
            _left += 1
        import os
        if os.getenv("KPRINT"):
            print(f"[emit] flushed {_left} leftover filler steps")
        for stl in (2, 3):
            for _ in oproj_gen(3, stl):
                pass

        if DEBUG:
            dbg = {
                "d_qT": qT_sb, "d_ktrepA": kT_sb, "d_ktrepB": kT_sw,
                "d_vT": vT_sb, "d_vtiles": v_tiles, "d_attnT": attn_T,
            }
            for name, t_ in dbg.items():
                dt_ = nc.dram_tensor(name, list(t_.shape), bf16,
                                     kind="ExternalOutput")
                nc.sync.dma_start(out=dt_[:], in_=t_[:])

    for pool in (expp, work, persist, wpool, consts):
        pool.release()


_NC_CACHE = None


def build_nc():
    global _NC_CACHE
    if _NC_CACHE is None:
        nc = bacc.Bacc("TRN2")
        with tile.TileContext(nc) as tc:
            _emit(tc)
        nc.compile()
        _NC_CACHE = nc
    return _NC_CACHE


def _bf16(a):
    return np.ascontiguousarray(np.asarray(a, dtype=np.float32)).astype(
        ml_dtypes.bfloat16)


def make_in_maps(hidden_state, Wq, bq, Wk, bk, Wv, bv, Wo):
    hidden_state = np.asarray(hidden_state, dtype=np.float32)
    Wq, Wk, Wv, Wo = (np.asarray(a, dtype=np.float32) for a in (Wq, Wk, Wv, Wo))
    bq, bk, bv = (np.asarray(a, dtype=np.float32) for a in (bq, bk, bv))
    htb = [_bf16(hidden_state[b].T) for b in range(B)]
    in_maps = []
    for core in range(NCORES):
        b, gs = divmod(core, GS)
        # wq: [HID, DQ] -> [NPAIR, P(part), KT, P(cols)]
        wqt = Wq[gs * DQ:(gs + 1) * DQ, :].T.reshape(KT, P, NPAIR, P)
        # wk/wv: [HID, DKV] -> [P(part), KT, DKV]
        wkt = Wk[gs * DKV:(gs + 1) * DKV, :].T.reshape(KT, P, DKV)
        wvt = Wv[gs * DKV:(gs + 1) * DKV, :].T.reshape(KT, P, DKV)
        in_maps.append({
            "ht": htb[b],
            "wq": _bf16(wqt.transpose(2, 1, 0, 3)),
            "wk": _bf16(wkt.transpose(1, 0, 2)),
            "wv": _bf16(wvt.transpose(1, 0, 2)),
            "wo": _bf16(Wo[:, gs * DQ:(gs + 1) * DQ].T),
            "bq": np.ascontiguousarray(bq[gs * DQ:(gs + 1) * DQ]),
            "bk": np.ascontiguousarray(bk[gs * DKV:(gs + 1) * DKV]),
            "bv": np.ascontiguousarray(bv[gs * DKV:(gs + 1) * DKV]),
        })
    return in_maps


def unshard(results, bo):
    bo = np.asarray(bo, dtype=np.float32)
    out = np.empty((B, S, HID), dtype=np.float32)
    for b in range(B):
        acc = np.zeros((S, HID), dtype=np.float64)
        for gs in range(GS):
            acc += np.asarray(results[b * GS + gs]["opart"], dtype=np.float32)
        out[b] = (acc + bo).astype(np.float32)
    return out


def kernel(hidden_state, attention_mask, Wq, bq, Wk, bk, Wv, bv, Wo, bo):
    # attention_mask is all-ones for this problem (fill: ones) -> identity.
    nc = build_nc()
    in_maps = make_in_maps(hidden_state, Wq, bq, Wk, bk, Wv, bv, Wo)
    res = run_bass_kernel_spmd(nc, in_maps, list(range(NCORES)))
    return unshard(res.results, bo)
